# revision 5
# baseline (speedup 1.0000x reference)
"""MemoryMoCo forward (out, probs, new_memory) on 8 Trainium2 NeuronCores.

Reference semantics (see problem):
    l_pos = mean_p(q[b] . k[pos(b,p)])                      [256,1]
    l_neg = q @ memory.T                                    [256,131072]
    out   = exp(concat([l_pos, l_neg], 1) / T)
    Z     = mean(out) * 1e6 ;  out = out / Z
    probs = mean(out[:,0] / sum(out, 1))
    new_memory = memory with rows 0..63 <- mean of each video's 4 clip keys

Sharding: memory bank split row-wise (queue dim) into 8 shards of 16384;
each core computes exp(q @ shard.T / T) for its columns, the rowsums are
AllReduced (1 KB payload) to form Z on every core, each core scales its
shard by 1/Z and writes it out.  q/k are replicated.  The memory-bank
scatter update is done on-device: every core streams its shard back out
(copy-through via SBUF), core 0 blends the first 64 rows with k_mean
(computed on-device from k) via a per-core 0/1 mask input so the SPMD
program is identical on all cores.
"""

import contextlib
import ctypes
import json
import sys
import types

import numpy as np

sys.path.insert(0, "/opt/trn_rl_repo")

import concourse.bass as bass
import concourse.mybir as mybir
import concourse.tile as tile
from concourse.tile import ScopedClock

# ---------------------------------------------------------------------------
# Environment shims (self-contained: no sibling imports).
# 1) `antenv.axon_hooks` NTFF profile hook so trace=True works under axon.
# 2) Tile tail-drain emits >1 sync waits; this walrus build allows only one
#    sync wait per instruction -> split them.
# 3) Same limit applies to every instruction kind: post-process the
#    serialized BIR to hoist extra waits onto single-wait Drain no-ops.
# ---------------------------------------------------------------------------


def _install_shims():
    if getattr(bass, "_moco_shims_installed", False):
        return

    def _ntff_profile_via_ctypes(so_path):
        try:
            lib = ctypes.CDLL(so_path)
        except OSError:
            return None
        if not hasattr(lib, "axon_start_nrt_profile"):
            return None
        lib.axon_start_nrt_profile.argtypes = [
            ctypes.POINTER(ctypes.c_int64),
            ctypes.c_size_t,
        ]
        lib.axon_start_nrt_profile.restype = ctypes.c_int64
        lib.axon_stop_nrt_profile.argtypes = [ctypes.c_char_p]
        lib.axon_stop_nrt_profile.restype = ctypes.c_int64

        @contextlib.contextmanager
        def _hook(output_dir, device_ids):
            import jax

            jax.devices()
            if device_ids:
                ids = (ctypes.c_int64 * len(device_ids))(*device_ids)
                rc = lib.axon_start_nrt_profile(ids, len(device_ids))
            else:
                rc = lib.axon_start_nrt_profile(None, 0)
            if rc != 0:
                raise RuntimeError(f"axon_start_nrt_profile rc={rc}")
            try:
                yield
            finally:
                n = lib.axon_stop_nrt_profile(str(output_dir).encode())
                if n < 0:
                    raise RuntimeError(f"axon_stop_nrt_profile rc={n}")

        return _hook

    if "antenv.axon_hooks" not in sys.modules:
        hook = _ntff_profile_via_ctypes("/opt/axon/libaxon_pjrt.so")
        mod = types.ModuleType("antenv.axon_hooks")
        mod.get_axon_ntff_profile_hook = lambda: hook
        mod.set_axon_ntff_profile_hook = lambda h: None
        sys.modules["antenv.axon_hooks"] = mod

    import concourse.bass_utils as bu

    bu.upload_artifacts = lambda tmpdir: str(tmpdir)

    def _drain_and_barrier_split(self, tick_clock, wait_clock):
        drain_inst = self.nc.sync.drain()
        wait_clock.add_sem_waits(
            drain_inst.ins, ScopedClock({None: tick_clock.global_clock})
        )
        si = drain_inst.ins.sync_info
        if si is not None and si.on_wait and len(si.on_wait) > 1:
            waits = list(si.on_wait)
            drain_inst.ins.sync_info = mybir.SyncInfo(
                on_wait=[waits[0]], on_update=si.on_update
            )
            for w in waits[1:]:
                extra = self.nc.sync.drain()
                extra.ins.sync_info = mybir.SyncInfo(on_wait=[w], on_update=[])
        self.nc.all_engine_barrier()
        assert self.sems is not None
        popped = self.nc._tile_sem_poison_stack.pop()
        assert popped is self._sem_poison
        self.nc.clear_and_free_semaphores(list(self.sems.allocated().values()))
        self.nc.all_engine_barrier()

    tile.TileContext._drain_and_barrier = _drain_and_barrier_split

    def _split_multiwait_bir_bytes(raw):
        bir = json.loads(raw)
        changed = False
        for fn in bir.get("functions", []):
            for blk in fn.get("blocks", []):
                out = []
                for inst in blk.get("instructions", []):
                    si = inst.get("sync_info")
                    waits = (si or {}).get("on_wait") or []
                    if len(waits) > 1:
                        changed = True
                        for kk, w in enumerate(waits[:-1]):
                            out.append({
                                "debug": inst.get("debug"),
                                "engine": inst["engine"],
                                "ins": [],
                                "outs": [],
                                "is_reset_sema": False,
                                "name": f"{inst['name']}-mw{kk}",
                                "opcode": "Drain",
                                "sync_info": {"on_update": [], "on_wait": [w]},
                            })
                        si["on_wait"] = [waits[-1]]
                    out.append(inst)
                blk["instructions"] = out
        return json.dumps(bir).encode() if changed else raw

    orig_to_json_bytes = bass.Bass.to_json_bytes
    bass.Bass.to_json_bytes = lambda self, *a, **kw: _split_multiwait_bir_bytes(
        orig_to_json_bytes(self, *a, **kw)
    )

    bass._moco_shims_installed = True


_install_shims()

# ---------------------------------------------------------------------------
# Problem constants (hardcoded per contest rules)
# ---------------------------------------------------------------------------
N_CORES = 8
BS = 256          # batch (rows of q/k)
D = 128           # feature dim
QK = 131072       # queue size
JS = QK // N_CORES  # 16384 queue rows per core
T = 0.07
OUTPUT_SIZE = 1000000
CLIPS = 4
BATCH = BS // CLIPS  # 64 videos

TPB = 16           # mem tiles per staging block
NB = JS // (TPB * 128)   # 8 staging blocks per core
NGP = JS // 1024         # 16 group-pairs (1024 queue rows each)

F32 = mybir.dt.float32
F32R = mybir.dt.float32r
EXP = mybir.ActivationFunctionType.Exp

_CACHED = {}


def _build():
    nc = bass.Bass("TRN2", target_bir_lowering=False, debug=False,
                   num_devices=N_CORES)

    q_in = nc.dram_tensor("q", [BS, D], F32, kind="ExternalInput").ap()
    k_in = nc.dram_tensor("k", [BS, D], F32, kind="ExternalInput").ap()
    mem_in = nc.dram_tensor("mem", [JS, D], F32, kind="ExternalInput").ap()
    smat_in = nc.dram_tensor("smat", [BS, BATCH], F32, kind="ExternalInput").ap()
    ones_in = nc.dram_tensor("ones", [D, D], F32, kind="ExternalInput").ap()
    ident_in = nc.dram_tensor("ident", [D, D], F32, kind="ExternalInput").ap()
    sel_in = nc.dram_tensor("sel", [BATCH, D], mybir.dt.int32, kind="ExternalInput").ap()

    oshard = nc.dram_tensor("oshard", [BS, JS], F32, kind="ExternalOutput").ap()
    nmem = nc.dram_tensor("nmem", [JS, D], F32, kind="ExternalOutput").ap()
    lpos_o = nc.dram_tensor("lpos", [128, 2], F32, kind="ExternalOutput").ap()
    probs_o = nc.dram_tensor("probs", [1, 1], F32, kind="ExternalOutput").ap()

    with tile.TileContext(nc) as tc:
        with (
            tc.tile_pool(name="stage", bufs=3) as stage,
            tc.tile_pool(name="memt", bufs=6) as memtp,
            tc.tile_pool(name="expbuf", bufs=1) as expbuf,
            tc.tile_pool(name="small", bufs=1) as small,
            tc.tile_pool(name="tp_ps", bufs=2, space="PSUM") as tp_ps,
            tc.tile_pool(name="glue_ps", bufs=2, space="PSUM") as glue_ps,
            tc.tile_pool(name="mm_ps", bufs=2, space="PSUM") as mm_ps,
            tc.tile_pool(name="dram", bufs=2, space="DRAM") as dram,
        ):
            # ---- small persistent tiles -----------------------------------
            q0 = small.tile([128, D], F32, tag="q0")
            q1 = small.tile([128, D], F32, tag="q1")
            k0 = small.tile([128, D], F32, tag="k0")
            k1 = small.tile([128, D], F32, tag="k1")
            s0 = small.tile([128, BATCH], F32, tag="s0")
            s1 = small.tile([128, BATCH], F32, tag="s1")
            ones = small.tile([D, D], F32, tag="ones")
            ident = small.tile([D, D], F32, tag="ident")
            sel = small.tile([BATCH, D], mybir.dt.int32, tag="sel")
            mem64 = small.tile([BATCH, D], F32, tag="mem64")
            qt = small.tile([128, BS], F32R, tag="qt")        # q^T (f32r)
            ks_sb = small.tile([128, BATCH], F32, tag="ks_sb")  # ksum^T fp32
            ks_r = small.tile([128, BATCH], F32R, tag="ks_r")   # ksum^T f32r
            kmean = small.tile([BATCH, D], F32, tag="kmean")
            blend = small.tile([BATCH, D], F32, tag="blend")
            partials = small.tile([128, 2 * NGP], F32, tag="partials")
            rsum = small.tile([128, 2], F32, tag="rsum")
            rstot = small.tile([128, 2], F32, tag="rstot")
            rtot = small.tile([128, 2], F32, tag="rtot")
            sd = small.tile([128, 2], F32, tag="sd")
            dvec = small.tile([128, 2], F32, tag="dvec")
            prodt = small.tile([128, D], F32, tag="prodt")
            maskt = small.tile([128, BATCH], F32, tag="maskt")
            explp = small.tile([128, 2], F32, tag="explp")
            lposc = small.tile([128, 2], F32, tag="lposc")
            rt1 = small.tile([128, 1], F32, tag="rt1")
            zsc = small.tile([1, 1], F32, tag="zsc")
            zinv = small.tile([1, 1], F32, tag="zinv")
            zb = small.tile([128, 1], F32, tag="zb")
            rcp = small.tile([128, 2], F32, tag="rcp")
            prv = small.tile([128, 2], F32, tag="prv")
            pr1 = small.tile([128, 1], F32, tag="pr1")
            probs_sb = small.tile([1, 1], F32, tag="probs_sb")

            # ---- constant / replicated input loads ------------------------
            nc.sync.dma_start(q0[:], q_in[0:128, :])
            nc.sync.dma_start(q1[:], q_in[128:256, :])
            nc.sync.dma_start(k0[:], k_in[0:128, :])
            nc.sync.dma_start(k1[:], k_in[128:256, :])
            nc.sync.dma_start(s0[:], smat_in[0:128, :])
            nc.sync.dma_start(s1[:], smat_in[128:256, :])
            nc.sync.dma_start(ones[:], ones_in[:])
            nc.sync.dma_start(ident[:], ident_in[:])
            nc.sync.dma_start(sel[:], sel_in[:])
            nc.sync.dma_start(mem64[:], mem_in[0:BATCH, :])

            # ---- q^T via PE transpose -> f32r -----------------------------
            qt_ps = glue_ps.tile([128, 512], F32, tag="g")
            nc.tensor.transpose(qt_ps[:, 0:128], q0[:], ident[:])
            nc.tensor.transpose(qt_ps[:, 128:256], q1[:], ident[:])
            nc.vector.tensor_copy(qt[:], qt_ps[:, 0:256])

            # ---- ksum^T = sum over clips: k^T @ S  ------------------------
            ks_ps = glue_ps.tile([128, 512], F32, tag="g")
            nc.tensor.matmul(ks_ps[:, 0:BATCH], k0[:], s0[:], start=True, stop=False)
            nc.tensor.matmul(ks_ps[:, 0:BATCH], k1[:], s1[:], start=False, stop=True)
            nc.vector.tensor_copy(ks_sb[:], ks_ps[:, 0:BATCH])
            nc.vector.tensor_copy(ks_r[:], ks_ps[:, 0:BATCH])

            # ---- k_mean rows + blended first-64 rows of new memory --------
            km_ps = glue_ps.tile([128, 512], F32, tag="g")
            nc.tensor.transpose(km_ps[0:BATCH, 0:D], ks_sb[:], ident[:])
            nc.scalar.mul(kmean[:], km_ps[0:BATCH, 0:D], 1.0 / CLIPS)
            nc.vector.select(blend[:], sel[:], kmean[:], mem64[:])
            nc.sync.dma_start(nmem[0:BATCH, :], blend[:])

            # ---- l_pos: s[b] = q[b].ksum[v(b)], d[b] = q[b].k[b] ----------
            for ch, (qc, kc, sc) in enumerate(((q0, k0, s0), (q1, k1, s1))):
                sq_ps = glue_ps.tile([128, 512], F32, tag="g")
                nc.tensor.matmul(
                    sq_ps[:, 0:BATCH], qt[:, ch * 128:(ch + 1) * 128], ks_r[:],
                    start=True, stop=True,
                )
                # diag extract: sum over v of sq * S
                nc.vector.tensor_mul(maskt[:], sq_ps[:, 0:BATCH], sc[:])
                nc.vector.reduce_sum(sd[:, ch:ch + 1], maskt[:],
                                     axis=mybir.AxisListType.X)
                nc.vector.tensor_mul(prodt[:], qc[:], kc[:])
                nc.vector.reduce_sum(dvec[:, ch:ch + 1], prodt[:],
                                     axis=mybir.AxisListType.X)
            nc.vector.tensor_sub(sd[:], sd[:], dvec[:])
            # exp(l_pos/T) = exp((s-d)/(3T))
            nc.scalar.activation(explp[:], sd[:], EXP, scale=1.0 / (3.0 * T))

            # ---- main loop: load shard, transpose, matmul, exp ------------
            mem_t = mem_in.rearrange("(b t p) d -> b p t d", t=TPB, p=128)
            nmem_t = nmem.rearrange("(b t p) d -> b p t d", t=TPB, p=128)

            mblocks = []
            for b in range(NB):
                mb = stage.tile([128, TPB * D], F32, tag="mb")
                mbv = mb[:].rearrange("p (t d) -> p t d", t=TPB)
                nc.sync.dma_start(mbv, mem_t[b])
                # copy-through to new memory (block 0: skip first 64 rows)
                if b == 0:
                    nc.sync.dma_start(nmem[BATCH:128, :], mb[BATCH:128, 0:D])
                    nc.sync.dma_start(nmem_t[0][:, 1:TPB, :], mbv[:, 1:TPB, :])
                else:
                    nc.sync.dma_start(nmem_t[b], mbv)
                mblocks.append(mb)

            exp_sb = expbuf.tile([128, 2 * JS], F32)

            # per group of 512 queue rows: 4 transposes -> memT tile
            memt_tiles = []
            for g in range(JS // 512):
                mb = mblocks[g // 4]
                tpt = tp_ps.tile([128, 512], F32, tag="tp")
                for t in range(4):
                    lt = (g % 4) * 4 + t
                    nc.tensor.transpose(
                        tpt[:, t * 128:(t + 1) * 128],
                        mb[:, lt * D:(lt + 1) * D],
                        ident[:],
                    )
                mt = memtp.tile([128, 512], F32R, tag="mt")
                nc.vector.tensor_copy(mt[:], tpt[:])
                memt_tiles.append(mt)

            # matmuls + exp, chunk-major inside each group pair
            for gp in range(NGP):
                for ch in range(2):
                    mm = mm_ps.tile([128, 1024], F32, tag="mm")
                    nc.tensor.matmul(
                        mm[:, 0:512], qt[:, ch * 128:(ch + 1) * 128],
                        memt_tiles[2 * gp][:], start=True, stop=True,
                    )
                    nc.tensor.matmul(
                        mm[:, 512:1024], qt[:, ch * 128:(ch + 1) * 128],
                        memt_tiles[2 * gp + 1][:], start=True, stop=True,
                    )
                    nc.scalar.activation(
                        exp_sb[:, ch * JS + gp * 1024: ch * JS + (gp + 1) * 1024],
                        mm[:],
                        EXP,
                        scale=1.0 / T,
                        accum_out=partials[:, ch * NGP + gp: ch * NGP + gp + 1],
                    )

            # ---- local rowsums + AllReduce --------------------------------
            nc.vector.reduce_sum(rsum[:, 0:1], partials[:, 0:NGP],
                                 axis=mybir.AxisListType.X)
            nc.vector.reduce_sum(rsum[:, 1:2], partials[:, NGP:2 * NGP],
                                 axis=mybir.AxisListType.X)

            cc_in = dram.tile([128, 2], F32)
            cc_out = dram.tile([128, 2], F32, addr_space="Shared")
            nc.sync.dma_start(cc_in[:], rsum[:])
            nc.gpsimd.collective_compute(
                "AllReduce",
                mybir.AluOpType.add,
                replica_groups=[list(range(N_CORES))],
                ins=[cc_in.opt()],
                outs=[cc_out.opt()],
            )
            nc.sync.dma_start(rstot[:], cc_out[:])

            # ---- Z, probs, l_pos column -----------------------------------
            nc.vector.tensor_add(rtot[:], rstot[:], explp[:])
            nc.vector.reduce_sum(rt1[:], rtot[:], axis=mybir.AxisListType.X)
            s_ps = glue_ps.tile([128, 512], F32, tag="g")
            nc.tensor.matmul(s_ps[0:1, 0:1], rt1[:], ones[:, 0:1],
                             start=True, stop=True)
            nc.scalar.mul(zsc[:], s_ps[0:1, 0:1],
                          float(OUTPUT_SIZE) / (BS * (QK + 1)))
            nc.vector.reciprocal(zinv[:], zsc[:])
            zb_ps = glue_ps.tile([128, 512], F32, tag="g")
            nc.tensor.matmul(zb_ps[:, 0:1], ones[0:1, :], zinv[:],
                             start=True, stop=True)
            nc.vector.tensor_copy(zb[:], zb_ps[:, 0:1])

            nc.vector.reciprocal(rcp[:], rtot[:])
            nc.vector.tensor_mul(prv[:], explp[:], rcp[:])
            nc.vector.reduce_sum(pr1[:], prv[:], axis=mybir.AxisListType.X)
            p_ps = glue_ps.tile([128, 512], F32, tag="g")
            nc.tensor.matmul(p_ps[0:1, 0:1], pr1[:], ones[:, 0:1],
                             start=True, stop=True)
            nc.scalar.mul(probs_sb[:], p_ps[0:1, 0:1], 1.0 / BS)
            nc.sync.dma_start(probs_o[:], probs_sb[:])

            nc.vector.tensor_scalar_mul(lposc[:], explp[:], zb[:])
            nc.sync.dma_start(lpos_o[:], lposc[:])

            # ---- scale shard by 1/Z and store -----------------------------
            for s in range(16):
                ch, js = divmod(s, 8)
                sl = exp_sb[:, s * 2048:(s + 1) * 2048]
                nc.vector.tensor_scalar_mul(sl, sl, zb[:])
                nc.sync.dma_start(
                    oshard[ch * 128:(ch + 1) * 128,
                           js * 2048:(js + 1) * 2048],
                    sl,
                )

    return nc


def _host_consts():
    b = np.arange(BS)
    smat = (b[:, None] % BATCH == np.arange(BATCH)[None, :]).astype(np.float32)
    ones = np.ones((D, D), np.float32)
    ident = np.eye(D, dtype=np.float32)
    return smat, ones, ident


def _get_nc():
    if "nc" not in _CACHED:
        _CACHED["nc"] = _build()
    return _CACHED["nc"]


def _run(q, k, memory, trace=False):
    from concourse.bass_utils import run_bass_kernel_spmd

    nc = _get_nc()
    smat, ones, ident = _host_consts()
    q = np.ascontiguousarray(q, np.float32)
    k = np.ascontiguousarray(k, np.float32)
    memory = np.ascontiguousarray(memory, np.float32)

    in_maps = []
    for c in range(N_CORES):
        sel = np.full((BATCH, D), 1 if c == 0 else 0, np.int32)
        in_maps.append({
            "q": q,
            "k": k,
            "mem": memory[c * JS:(c + 1) * JS],
            "smat": smat,
            "ones": ones,
            "ident": ident,
            "sel": sel,
        })
    res = run_bass_kernel_spmd(nc, in_maps, list(range(N_CORES)), trace=trace)

    out = np.empty((BS, QK + 1), np.float32)
    out[:, 0] = res.results[0]["lpos"].T.reshape(BS)
    for c in range(N_CORES):
        out[:, 1 + c * JS:1 + (c + 1) * JS] = res.results[c]["oshard"]
    new_memory = np.concatenate(
        [res.results[c]["nmem"] for c in range(N_CORES)], axis=0
    )
    probs = np.float32(res.results[0]["probs"][0, 0])
    return (out, probs, new_memory), res


def kernel(q, k, memory, i=0, **_unused):
    (out, probs, new_memory), _ = _run(q, k, memory, trace=False)
    return out, probs, new_memory


def kernel_profiled(q, k, memory, i=0, **_unused):
    """Same as kernel() but with NTFF tracing; returns (outputs, results)."""
    return _run(q, k, memory, trace=True)
